# revision 8
# baseline (speedup 1.0000x reference)
"""Trainium2 Bass kernel for nn_CustomAttentionLayer (sparse_attention).

Strategy (8 NeuronCores, SPMD single launch):
 - Shard the K=1024 query-cluster axis: core m owns rows [128m, 128m+128).
 - Phase 1 (DMA-bound, ~67MB/core): stream the per-core column slices of
   q_assignments / k_assignments (fp32 in HBM, cast to fp16 on the fly)
   through the PE array against the N-side feature matrix
   X = [one_hot(iso) | ones | coords] to get the (16,128) reductions
   (d_k_raw.T, sum, centroid numerators) for both q and k sides.
 - The k-side (16,128) slab is AllGather'd across the 8 cores (61KB) so every
   core has the full k-side (16,1024).
 - Phase 2: R = G*H computed with a (4i x 32c)-partition packing: the G/H MLP
   hidden activations are built with per-partition scalar bias/scale tricks,
   relu'd on ACT/DVE/GPSIMD, and contracted over the hidden channel axis on
   the PE array via constant sign-pattern matmuls that accumulate straight
   into (128,1024) G_inner / H_inner psum tiles. Attention + FFN + layernorms
   finish on-chip; each core writes its (128,256) slab of the output.
"""
import numpy as np

import concourse.bass as bass
import concourse.mybir as mybir
import concourse.tile as tile
from concourse.bass_utils import run_bass_kernel_spmd

F32 = mybir.dt.float32
F16 = mybir.dt.float16
AF = mybir.ActivationFunctionType
OP = mybir.AluOpType

NCORES = 8
K, D, N, NISO = 1024, 256, 65536, 12
KSL = K // NCORES           # 128 rows per core
NCH = N // 128              # 512 contraction chunks
GRP = 32                    # chunks per DMA group
NGRP = NCH // GRP           # 16
XC = 16                     # X feature cols: [oh(12) | ones | cx | cy]  (+0 pad)

_cache = {}


# ---------------------------------------------------------------------------
# walrus in this container rejects >1 sync wait per instruction; split extras
# onto single-wait NOPs on the same engine right before the instruction.
def _split_multiwaits(nc):
    ctr = 0
    for f in nc.m.functions:
        for bb in f.blocks:
            for inst in list(bb.instructions):
                si = inst.sync_info
                if si is None:
                    continue
                waits = list(si.on_wait)
                if len(waits) <= 1:
                    continue
                si.on_wait = [waits[-1]]
                pos = None
                for j, cur in enumerate(bb.instructions):
                    if cur.name == inst.name:
                        pos = j
                        break
                assert pos is not None
                for k2, w in enumerate(waits[:-1]):
                    nop = mybir.InstNoOp(
                        name=f"wsplit-{ctr}",
                        sync_info=mybir.SyncInfo(on_wait=[w], on_update=[]),
                        engine=inst.engine,
                        bass_nofuse=True,
                    )
                    ctr += 1
                    nc.register_instruction(nop)
                    bb.instructions.insert(pos + k2, nop)
    return ctr


def build_program():
    nc = bass.Bass()

    # ---------------- DRAM I/O ----------------
    def din(name, shape, dt=F32):
        return nc.dram_tensor(name, list(shape), dt, kind="ExternalInput")

    qa_t = din("qa_t", (128, NCH, 128))          # fp32, per-core A_q slice (transposed-chunked)
    ka_t = din("ka_t", (128, NCH, 128))
    qx_t = din("qx_t", (128, NCH, XC), F16)      # N-side features, fp16
    kx_t = din("kx_t", (128, NCH, XC), F16)
    qT3 = din("qT3", (128, 2, 128), F16)         # query[sl].T chunked
    keyT3 = din("keyT3", (128, 2, 1024), F16)    # key.T chunked
    key3 = din("key3", (128, 8, 256), F16)       # key chunked
    q_sb_h = din("q_sbh", (128, 256))            # query[sl] fp32
    g1ab_h = din("g1ab", (13, 32))               # [G1A * |w2| ; g1_b * |w2|]
    g1b4_h = din("g1b4", (12, 128))              # G1B * |w2| tiled x4
    cc_g_h = din("cc_g", (128, 256), F16)        # sign-pattern for G contraction
    cc_h_h = din("cc_h", (128, 256), F16)
    bc32_h = din("bc32", (128, 8, 128), F16)      # row-broadcast patterns
    wq_h = din("wq_h", (1, 32))                  # h1_w[1]*|h2w|
    b1_h = din("b1_h", (1, 32))                  # h1_b*|h2w|
    wk4_h = din("wk4", (1, 128))                 # h1_w[2]*|h2w| tiled x4
    aH4_h = din("aH4", (128, 1))                 # h1_w[0]*|h2w| tiled x4 (col)
    g2b_h = din("g2b", (128, 1))
    h2b_h = din("h2b", (128, 1))
    ones_h = din("ones_r", (1, 128))
    sens_h = din("sens", (1, 2))
    sensr_h = din("sensr", (128, 2))
    i32_h = din("i128f", (128, 128))             # fp32 identity
    i16_h = din("i128h", (128, 128), F16)        # fp16 identity
    f1w_h = din("f1w", (128, 2, 8, 128), F16)
    f1b_h = din("f1b", (128, 8))
    f2w_h = din("f2w", (128, 8, 256), F16)
    f2b_h = din("f2br", (128, 256))
    l1g_h = din("l1g", (128, 256))
    l1b_h = din("l1b", (128, 256))
    l2g_h = din("l2g", (128, 256))
    l2b_h = din("l2b", (128, 256))
    eps_h = din("epsc", (128, 1))

    out_d = nc.dram_tensor("out", [128, 256], F32, kind="ExternalOutput")

    kside_d = nc.dram_tensor("kside", [16, 128], F32)
    kall_d = nc.dram_tensor("kall", [128, 128], F32, addr_space="Shared")

    with tile.TileContext(nc) as tc:
        with (
            tc.tile_pool(name="consts", bufs=1) as cp,
            tc.tile_pool(name="ph1", bufs=3) as p1,
            tc.tile_pool(name="sb", bufs=1) as sp,
            tc.tile_pool(name="sb2", bufs=2) as sp2,
            tc.tile_pool(name="pp", bufs=2, space="PSUM") as pp,
        ):
            # ---------------- const loads ----------------
            def cload(dram, dt=None, tag=None):
                t = cp.tile(list(dram.shape), dt or dram.dtype, tag=tag or dram.name)
                nc.sync.dma_start(out=t[:], in_=dram[:])
                return t

            qT3_s = cload(qT3)
            keyT3_s = cload(keyT3)
            key3_s = cload(key3)
            qsbh_s = cload(q_sb_h)
            g1ab_s = cload(g1ab_h)
            g1b4_s = cload(g1b4_h)
            ccg_s = cload(cc_g_h)
            cch_s = cload(cc_h_h)
            bc32_s = cload(bc32_h)
            wq_s = cload(wq_h)
            b1_s = cload(b1_h)
            wk4_s = cload(wk4_h)
            aH4_s = cload(aH4_h)
            g2b_s = cload(g2b_h)
            h2b_s = cload(h2b_h)
            ones_s = cload(ones_h)
            sens_s = cload(sens_h)
            sensr_s = cload(sensr_h)
            i32_s = cload(i32_h)
            i16_s = cload(i16_h)
            f1w_s = cload(f1w_h)
            f1b_s = cload(f1b_h)
            f2w_s = cload(f2w_h)
            f2b_s = cload(f2b_h)
            l1g_s = cload(l1g_h)
            l1b_s = cload(l1b_h)
            l2g_s = cload(l2g_h)
            l2b_s = cload(l2b_h)
            eps_s = cload(eps_h)

            # ---------------- phase 1: big reductions ----------------
            def big_reduce(a_dram, x_dram, ps, nm):
                for g in range(NGRP):
                    at = p1.tile([128, GRP, 128], F16, tag=f"a_{nm}")
                    nc.gpsimd.dma_start(out=at[:], in_=a_dram[:, g * GRP:(g + 1) * GRP, :])
                    xt = p1.tile([128, GRP, XC], F16, tag=f"x_{nm}")
                    nc.sync.dma_start(out=xt[:], in_=x_dram[:, g * GRP:(g + 1) * GRP, :])
                    for c in range(GRP):
                        nc.tensor.matmul(
                            ps[:], lhsT=xt[:, c, :], rhs=at[:, c, :],
                            start=(g == 0 and c == 0), stop=(g == NGRP - 1 and c == GRP - 1),
                        )

            # k first: its collective exchange overlaps the q reduction
            psk = pp.tile([16, 128], F32, tag="gh")
            big_reduce(ka_t, kx_t, psk, "k")

            # exchange k-side
            ksb = sp.tile([16, 128], F32, tag="ksb")
            nc.scalar.activation(ksb[:], psk[:], AF.Copy)
            nc.sync.dma_start(out=kside_d[:], in_=ksb[:])
            nc.gpsimd.collective_compute(
                "AllGather", OP.bypass,
                replica_groups=[list(range(NCORES))],
                ins=[kside_d[:]],
                outs=[kall_d[:]],
            )
            kview = kall_d.rearrange("(g c) k -> c g k", c=16)
            dkpT = sp.tile([12, 1024], F32, tag="dkpT")
            nc.sync.dma_start(out=dkpT[:].rearrange("c (g k) -> c g k", g=8),
                              in_=kview[0:12, :, :])
            ksum_r = sp.tile([1, 1024], F32, tag="ksum_r")
            nc.sync.dma_start(out=ksum_r[:].rearrange("c (g k) -> c g k", g=8),
                              in_=kview[12:13, :, :])
            kcx_r = sp.tile([1, 1024], F32, tag="kcx_r")
            nc.sync.dma_start(out=kcx_r[:].rearrange("c (g k) -> c g k", g=8),
                              in_=kview[13:14, :, :])
            kcy_r = sp.tile([1, 1024], F32, tag="kcy_r")
            nc.sync.dma_start(out=kcy_r[:].rearrange("c (g k) -> c g k", g=8),
                              in_=kview[14:15, :, :])

            # ---- k-side setup (overlaps q big_reduce) ----
            # rk = 1/(ksum+eps)  in place
            nc.vector.tensor_scalar_add(ksum_r[:], ksum_r[:], 1e-6)
            nc.vector.reciprocal(ksum_r[:], ksum_r[:])
            # centroids in place
            nc.vector.tensor_mul(kcx_r[:], kcx_r[:], ksum_r[:])
            nc.vector.tensor_mul(kcy_r[:], kcy_r[:], ksum_r[:])
            # nkps row
            s1 = sp.tile([1, 1024], F32, tag="s1")
            s2 = sp.tile([1, 1024], F32, tag="s2")
            nc.vector.tensor_scalar_sub(s1[:], kcx_r[:], sens_s[0:1, 0:1])
            nc.vector.tensor_scalar_sub(s2[:], kcy_r[:], sens_s[0:1, 1:2])
            nc.vector.tensor_mul(s1[:], s1[:], s1[:])
            nc.vector.tensor_mul(s2[:], s2[:], s2[:])
            nc.vector.tensor_add(s1[:], s1[:], s2[:])
            nkps_r = sp.tile([1, 1024], F32, tag="nkps_r")
            nc.scalar.activation(nkps_r[:], s1[:], AF.Sqrt)
            # dkp normalized: dkpT_n = dkpT * (ones x rk)
            rkb = pp.tile([12, 1024], F32, tag="wide")
            for b in range(2):
                nc.tensor.matmul(rkb[:, 512 * b:512 * (b + 1)], lhsT=ones_s[0:1, 0:12],
                                 rhs=ksum_r[0:1, 512 * b:512 * (b + 1)], start=True, stop=True)
            dkpn = sp.tile([12, 1024], F32, tag="dkpn")
            nc.vector.tensor_mul(dkpn[:], dkpT[:], rkb[:])
            # B4 = (G1B'|w2|).T @ dkp_n   -> (128, 1024)
            psB = pp.tile([128, 1024], F32, tag="wide")
            for b in range(2):
                nc.tensor.matmul(psB[:, 512 * b:512 * (b + 1)], lhsT=g1b4_s[:],
                                 rhs=dkpn[:, 512 * b:512 * (b + 1)], start=True, stop=True)
            B4 = sp.tile([128, 1024], F32, tag="B4")
            nc.scalar.activation(B4[:], psB[:], AF.Copy)
            # T24 = wk4 x nkps  -> sbuf (128,1024)
            psT2 = pp.tile([128, 1024], F32, tag="wide")
            for b in range(2):
                nc.tensor.matmul(psT2[:, 512 * b:512 * (b + 1)], lhsT=wk4_s[:],
                                 rhs=nkps_r[0:1, 512 * b:512 * (b + 1)], start=True, stop=True)
            T24 = sp.tile([128, 1024], F32, tag="T24")
            nc.scalar.activation(T24[:], psT2[:], AF.Copy)

            # q reduction second
            psq = pp.tile([16, 128], F32, tag="gh")
            big_reduce(qa_t, qx_t, psq, "q")

            # logits_raw = (q @ key.T)/sqrt(D)  (fp16 matmul, emitted late but
            # only depends on const tiles; scheduler fits it into phase 1)
            psl = pp.tile([128, 1024], F32, tag="wide")
            for c in range(2):
                for b in range(2):
                    nc.tensor.matmul(psl[:, 512 * b:512 * (b + 1)], lhsT=qT3_s[:, c, :],
                                     rhs=keyT3_s[:, c, 512 * b:512 * (b + 1)],
                                     start=(c == 0), stop=(c == 1))
            lraw = sp.tile([128, 1024], F32, tag="lraw")
            nc.scalar.activation(lraw[:], psl[:], AF.Copy, scale=1.0 / 16.0)

            # ---- q-side setup ----
            qsb = sp.tile([16, 128], F32, tag="qsb")
            nc.scalar.activation(qsb[:], psq[:], AF.Copy)
            pqT = pp.tile([128, 16], F32, tag="gh")
            nc.tensor.transpose(pqT[:], qsb[:], i32_s[0:16, 0:16])
            qT = sp.tile([128, 16], F32, tag="qT")
            nc.vector.tensor_copy(qT[:], pqT[:])
            rq_c = sp.tile([128, 1], F32, tag="rq_c")
            nc.vector.tensor_scalar_add(rq_c[:], qT[:, 12:13], 1e-6)
            nc.vector.reciprocal(rq_c[:], rq_c[:])
            qc2 = sp.tile([128, 2], F32, tag="qc2")
            nc.vector.tensor_scalar_mul(qc2[:], qT[:, 13:15], rq_c[:])
            nqc2 = sp.tile([128, 2], F32, tag="nqc2")
            nc.vector.tensor_scalar_mul(nqc2[:], qc2[:], -1.0)
            # n_ks col
            d2 = sp.tile([128, 2], F32, tag="d2")
            nc.vector.tensor_sub(d2[:], qc2[:], sensr_s[:])
            nc.vector.tensor_mul(d2[:], d2[:], d2[:])
            nks_c = sp.tile([128, 1], F32, tag="nks_c")
            nc.vector.tensor_reduce(nks_c[:], d2[:], mybir.AxisListType.X, OP.add)
            nc.scalar.activation(nks_c[:], nks_c[:], AF.Sqrt)
            # rows: rq_row, nks_row
            prow = pp.tile([1, 128], F32, tag="gh")
            nc.tensor.transpose(prow[:], rq_c[:], i32_s[:])
            rq_r = sp.tile([1, 128], F32, tag="rq_r")
            nc.vector.tensor_copy(rq_r[:], prow[:])
            prow2 = pp.tile([1, 128], F32, tag="gh")
            nc.tensor.transpose(prow2[:], nks_c[:], i32_s[:])
            nks_r = sp.tile([1, 128], F32, tag="nks_r")
            nc.vector.tensor_copy(nks_r[:], prow2[:])
            # normalized qsb rows 0:13 (row 12 = qsum*rq = 1 -> bias row)
            rqb = pp.tile([13, 128], F32, tag="gh")
            nc.tensor.matmul(rqb[:], lhsT=ones_s[0:1, 0:13], rhs=rq_r[:], start=True, stop=True)
            qsbn = sp.tile([13, 128], F32, tag="qsbn")
            nc.vector.tensor_mul(qsbn[:], qsb[0:13, :], rqb[:])
            # AT4 (128, 32): packed A'' bias
            psA = pp.tile([128, 32], F32, tag="gh")
            for ii in range(4):
                nc.tensor.matmul(psA[32 * ii:32 * (ii + 1), :], lhsT=g1ab_s[:],
                                 rhs=qsbn[:, ii::4], start=True, stop=True,
                                 tile_position=(0, 32 * ii))
            AT4 = sp.tile([128, 32], F32, tag="AT4")
            nc.vector.tensor_copy(AT4[:], psA[:])
            # T14 (128, 32): packed H bias
            psT1 = pp.tile([128, 32], F32, tag="gh")
            for ii in range(4):
                nc.tensor.matmul(psT1[32 * ii:32 * (ii + 1), :], lhsT=wq_s[:],
                                 rhs=nks_r[0:1, ii::4], start=True, stop=False,
                                 tile_position=(0, 32 * ii))
                nc.tensor.matmul(psT1[32 * ii:32 * (ii + 1), :], lhsT=b1_s[:],
                                 rhs=ones_s[0:1, ii::4], start=False, stop=True,
                                 tile_position=(0, 32 * ii))
            T14 = sp.tile([128, 32], F32, tag="T14")
            nc.vector.tensor_copy(T14[:], psT1[:])
            # n_kk (fp16): sqrt((kcx-qcx)^2 + (kcy-qcy)^2), i on partitions
            pKC = pp.tile([128, 1024], F32, tag="wide")
            for b in range(2):
                nc.tensor.matmul(pKC[:, 512 * b:512 * (b + 1)], lhsT=ones_s[:],
                                 rhs=kcx_r[0:1, 512 * b:512 * (b + 1)], start=True, stop=True)
            dx2 = sp.tile([128, 1024], F32, tag="dx2")
            nc.scalar.activation(dx2[:], pKC[:], AF.Square, bias=nqc2[:, 0:1])
            pKC2 = pp.tile([128, 1024], F32, tag="wide")
            for b in range(2):
                nc.tensor.matmul(pKC2[:, 512 * b:512 * (b + 1)], lhsT=ones_s[:],
                                 rhs=kcy_r[0:1, 512 * b:512 * (b + 1)], start=True, stop=True)
            dy2 = sp.tile([128, 1024], F32, tag="dy2")
            nc.scalar.activation(dy2[:], pKC2[:], AF.Square, bias=nqc2[:, 1:2])
            nc.vector.tensor_add(dx2[:], dx2[:], dy2[:])
            nkk16 = sp.tile([128, 1024], F16, tag="nkk16")
            nc.scalar.activation(nkk16[:], dx2[:], AF.Sqrt)

            # ---------------- phase 2 group loop ----------------
            gin = pp.tile([128, 1024], F32, tag="gh")
            hin = pp.tile([128, 1024], F32, tag="gh")
            # engine for the G-relu per group: spread across GPSIMD/ACT/DVE
            for g in range(32):
                b32 = g // 8
                pat = g % 8
                nkk4 = pp.tile([128, 1024], F32, tag="wide")
                for b in range(2):
                    nc.tensor.matmul(nkk4[:, 512 * b:512 * (b + 1)],
                                     lhsT=bc32_s[32 * b32:32 * (b32 + 1), pat, :],
                                     rhs=nkk16[32 * b32:32 * (b32 + 1), 512 * b:512 * (b + 1)],
                                     start=True, stop=True,
                                     tile_position=(32 * b32, 0))
                uh = sp2.tile([128, 1024], F32, tag="uh")
                nc.vector.scalar_tensor_tensor(uh[:], nkk4[:], aH4_s[:], T24[:],
                                               op0=OP.mult, op1=OP.add)
                ph = sp2.tile([128, 1024], F16, tag="ph")
                nc.scalar.activation(ph[:], uh[:], AF.Relu, bias=T14[:, g:g + 1])
                pg = sp2.tile([128, 1024], F16, tag="pg")
                if g % 4 == 3:
                    nc.vector.tensor_scalar(pg[:], B4[:], AT4[:, g:g + 1], 0.0, OP.add, OP.max)
                else:
                    nc.gpsimd.tensor_scalar(pg[:], B4[:], AT4[:, g:g + 1], 0.0, OP.add, OP.max)
                off = 124 - 4 * g
                for b in range(2):
                    nc.tensor.matmul(hin[:, 512 * b:512 * (b + 1)],
                                     lhsT=cch_s[:, off:off + 128],
                                     rhs=ph[:, 512 * b:512 * (b + 1)],
                                     start=(g == 0), stop=(g == 31))
                    nc.tensor.matmul(gin[:, 512 * b:512 * (b + 1)],
                                     lhsT=ccg_s[:, off:off + 128],
                                     rhs=pg[:, 512 * b:512 * (b + 1)],
                                     start=(g == 0), stop=(g == 31))

            # ---------------- tail: R, softmax, attention, FFN, LN ----------------
            rg = sp.tile([128, 1024], F32, tag="rg")
            nc.scalar.activation(rg[:], gin[:], AF.Relu, bias=g2b_s[:])
            rh = sp.tile([128, 1024], F32, tag="rh")
            nc.vector.tensor_scalar(rh[:], hin[:], h2b_s[:], 0.0, OP.add, OP.max)
            lg = sp.tile([128, 1024], F32, tag="lg")
            nc.vector.tensor_mul(lg[:], lraw[:], rg[:])
            nc.vector.tensor_mul(lg[:], lg[:], rh[:])
            mx = sp.tile([128, 1], F32, tag="mx")
            nc.vector.tensor_reduce(mx[:], lg[:], mybir.AxisListType.X, OP.max)
            nmx = sp.tile([128, 1], F32, tag="nmx")
            nc.vector.tensor_scalar_mul(nmx[:], mx[:], -1.0)
            pexp = sp.tile([128, 1024], F16, tag="pexp")
            sume = sp.tile([128, 1], F32, tag="sume")
            nc.scalar.activation(pexp[:], lg[:], AF.Exp, bias=nmx[:], accum_out=sume[:])
            rsum = sp.tile([128, 1], F32, tag="rsum")
            nc.vector.reciprocal(rsum[:], sume[:])
            # transpose pexp -> (128, 8, 128)
            pT = sp.tile([128, 8, 128], F16, tag="pT")
            for t in range(8):
                ptp = pp.tile([128, 128], F16, tag="wide")
                nc.tensor.transpose(ptp[:], pexp[:, 128 * t:128 * (t + 1)], i16_s[:])
                nc.vector.tensor_copy(pT[:, t, :], ptp[:])
            attn = pp.tile([128, 256], F32, tag="gh")
            for t in range(8):
                nc.tensor.matmul(attn[:], lhsT=pT[:, t, :], rhs=key3_s[:, t, :],
                                 start=(t == 0), stop=(t == 7))
            xpre = sp.tile([128, 256], F32, tag="xpre")
            nc.vector.scalar_tensor_tensor(xpre[:], attn[:], rsum[:], qsbh_s[:],
                                           op0=OP.mult, op1=OP.add)

            def layer_norm(src, gt, bt, tag):
                m = sp.tile([128, 1], F32, tag=f"m_{tag}")
                nc.vector.tensor_reduce(m[:], src[:], mybir.AxisListType.X, OP.add)
                nc.vector.tensor_scalar_mul(m[:], m[:], -1.0 / 256.0)
                xc = sp.tile([128, 256], F32, tag=f"xc_{tag}")
                nc.vector.tensor_scalar_add(xc[:], src[:], m[:])
                var = sp.tile([128, 1], F32, tag=f"v_{tag}")
                xc2 = sp.tile([128, 256], F32, tag=f"xc2_{tag}")
                nc.vector.scalar_tensor_tensor(xc2[:], xc[:], 1.0, xc[:],
                                               op0=OP.mult, op1=OP.mult, accum_out=var[:])
                lnv = sp.tile([128, 1], F32, tag=f"lv_{tag}")
                nc.scalar.activation(lnv[:], var[:], AF.Ln, scale=1.0 / 256.0, bias=eps_s[:])
                rstd = sp.tile([128, 1], F32, tag=f"rs_{tag}")
                nc.scalar.activation(rstd[:], lnv[:], AF.Exp, scale=-0.5)
                y = sp.tile([128, 256], F32, tag=f"y_{tag}")
                nc.vector.scalar_tensor_tensor(y[:], xc[:], rstd[:], gt[:],
                                               op0=OP.mult, op1=OP.mult)
                nc.vector.tensor_add(y[:], y[:], bt[:])
                return y

            x1 = layer_norm(xpre, l1g_s, l1b_s, "ln1")
            # FFN
            x1h = sp.tile([128, 256], F16, tag="x1h")
            nc.vector.tensor_copy(x1h[:], x1[:])
            xT = sp.tile([128, 2, 128], F16, tag="xT")
            for c in range(2):
                pxT = pp.tile([128, 128], F16, tag="wide")
                nc.tensor.transpose(pxT[:], x1h[:, 128 * c:128 * (c + 1)], i16_s[:])
                nc.vector.tensor_copy(xT[:, c, :], pxT[:])
            hT = sp.tile([128, 8, 128], F16, tag="hT")
            for t in range(8):
                psh = pp.tile([128, 128], F32, tag="wide")
                for c in range(2):
                    nc.tensor.matmul(psh[:], lhsT=f1w_s[:, c, t, :], rhs=xT[:, c, :],
                                     start=(c == 0), stop=(c == 1))
                nc.scalar.activation(hT[:, t, :], psh[:], AF.Relu, bias=f1b_s[:, t:t + 1])
            pso = pp.tile([128, 256], F32, tag="gh")
            for t in range(8):
                nc.tensor.matmul(pso[:], lhsT=hT[:, t, :], rhs=f2w_s[:, t, :],
                                 start=(t == 0), stop=(t == 7))
            y2 = sp.tile([128, 256], F32, tag="y2")
            nc.vector.tensor_add(y2[:], pso[:], x1[:])
            nc.vector.tensor_add(y2[:], y2[:], f2b_s[:])
            x2 = layer_norm(y2, l2g_s, l2b_s, "ln2")
            nc.sync.dma_start(out=out_d[:], in_=x2[:])

    _split_multiwaits(nc)
    return nc


# ---------------------------------------------------------------------------
def prep_inputs(inp):
    f32 = np.float32
    f16 = np.float16
    q_asn = np.asarray(inp["q_assignments"], f32)
    k_asn = np.asarray(inp["k_assignments"], f32)
    query = np.asarray(inp["query"], f32)
    key = np.asarray(inp["key_emb"], f32)

    def xfeat(coords, iso):
        oh = np.zeros((N, NISO), f32)
        oh[np.arange(N), np.asarray(iso) - 1] = 1.0
        x = np.concatenate([oh, np.ones((N, 1), f32), np.asarray(coords, f32)], axis=1)
        assert x.shape[1] == 15
        x = np.concatenate([x, np.zeros((N, 1), f32)], axis=1)  # pad to 16
        return np.ascontiguousarray(x.reshape(NCH, 128, XC).transpose(1, 0, 2)).astype(f16)

    qx_t = xfeat(inp["q_coords"], inp["q_iso"])
    kx_t = xfeat(inp["k_coords"], inp["k_iso"])

    g1 = np.asarray(inp["g1_w"], f32)          # (24, 32)
    g1b = np.asarray(inp["g1_b"], f32)         # (32,)
    g2 = np.asarray(inp["g2_w"], f32)[:, 0]    # (32,)
    g2b = float(np.asarray(inp["g2_b"], f32)[0])
    h1 = np.asarray(inp["h1_w"], f32)          # (3, 32)
    h1b = np.asarray(inp["h1_b"], f32)
    h2 = np.asarray(inp["h2_w"], f32)[:, 0]
    h2b = float(np.asarray(inp["h2_b"], f32)[0])

    aw2 = np.abs(g2)
    sg2 = np.sign(g2).astype(f32)
    aw2h = np.abs(h2)
    sh2 = np.sign(h2).astype(f32)

    g1ab = np.concatenate([g1[:12] * aw2[None, :], (g1b * aw2)[None, :]], axis=0)  # (13,32)
    g1b4 = np.tile(g1[12:] * aw2[None, :], (1, 4))                                  # (12,128)

    def ccpat(sgn):
        cc = np.zeros((128, 256), f32)
        for ii in range(4):
            for c in range(32):
                cc[32 * ii + c, 124 + ii] = sgn[c]
        return cc.astype(f16)

    cc_g = ccpat(sg2)
    cc_h = ccpat(sh2)

    bc32 = np.zeros((32, 8, 128), f32)
    for pat in range(8):
        for ii in range(4):
            for c in range(32):
                bc32[4 * pat + ii, pat, 32 * ii + c] = 1.0
    bc32 = np.tile(bc32, (4, 1, 1)).astype(f16)

    wq = (h1[1] * aw2h)[None, :].astype(f32)       # (1,32)
    b1r = (h1b * aw2h)[None, :].astype(f32)
    wk4 = np.tile(h1[2] * aw2h, 4)[None, :].astype(f32)   # (1,128)
    aH4 = np.tile(h1[0] * aw2h, 4)[:, None].astype(f32)   # (128,1)

    sens = np.asarray(inp["sensor_coords"], f32)[None, :]          # (1,2)
    sensr = np.tile(sens, (128, 1))                                 # (128,2)

    f1wt = np.ascontiguousarray(
        np.asarray(inp["ffn1_w"], f32).reshape(2, 128, 8, 128).transpose(1, 0, 2, 3)).astype(f16)
    f1b = np.ascontiguousarray(np.asarray(inp["ffn1_b"], f32).reshape(8, 128).T)
    f2wt = np.ascontiguousarray(
        np.asarray(inp["ffn2_w"], f32).reshape(8, 128, 256).transpose(1, 0, 2)).astype(f16)
    f2br = np.tile(np.asarray(inp["ffn2_b"], f32)[None, :], (128, 1))
    l1g = np.tile(np.asarray(inp["ln1_g"], f32)[None, :], (128, 1))
    l1b = np.tile(np.asarray(inp["ln1_b"], f32)[None, :], (128, 1))
    l2g = np.tile(np.asarray(inp["ln2_g"], f32)[None, :], (128, 1))
    l2b = np.tile(np.asarray(inp["ln2_b"], f32)[None, :], (128, 1))

    keyT3 = np.ascontiguousarray(key.T.reshape(2, 128, 1024).transpose(1, 0, 2)).astype(f16)
    key3 = np.ascontiguousarray(key.reshape(8, 128, 256).transpose(1, 0, 2)).astype(f16)

    shared = {
        "qx_t": qx_t, "kx_t": kx_t,
        "keyT3": keyT3, "key3": key3,
        "g1ab": g1ab, "g1b4": g1b4, "cc_g": cc_g, "cc_h": cc_h, "bc32": bc32,
        "wq_h": wq, "b1_h": b1r, "wk4": wk4, "aH4": aH4,
        "g2b": np.full((128, 1), g2b, f32), "h2b": np.full((128, 1), h2b, f32),
        "ones_r": np.ones((1, 128), f32),
        "sens": sens, "sensr": sensr,
        "i128f": np.eye(128, dtype=f32), "i128h": np.eye(128, dtype=f16),
        "f1w": f1wt, "f1b": f1b, "f2w": f2wt, "f2br": f2br,
        "l1g": l1g, "l1b": l1b, "l2g": l2g, "l2b": l2b,
        "epsc": np.full((128, 1), 1e-6, f32),
    }

    in_maps = []
    for m in range(NCORES):
        sl = slice(m * KSL, (m + 1) * KSL)
        qa = np.ascontiguousarray(
            q_asn[:, sl].reshape(NCH, 128, 128).transpose(1, 0, 2))
        ka = np.ascontiguousarray(
            k_asn[:, sl].reshape(NCH, 128, 128).transpose(1, 0, 2))
        qT3 = np.ascontiguousarray(
            query[sl].T.reshape(2, 128, 128).transpose(1, 0, 2)).astype(f16)
        im = dict(shared)
        im.update({
            "qa_t": qa, "ka_t": ka, "qT3": qT3,
            "q_sbh": np.ascontiguousarray(query[sl]),
        })
        in_maps.append(im)
    return in_maps


def kernel(**inputs) -> np.ndarray:
    if "nc" not in _cache:
        _cache["nc"] = build_program()
    nc = _cache["nc"]
    in_maps = prep_inputs(inputs)
    res = run_bass_kernel_spmd(nc, in_maps, list(range(NCORES)))
    return np.concatenate([res.results[m]["out"] for m in range(NCORES)], axis=0)
